# revision 13
# baseline (speedup 1.0000x reference)
"""BitLinear (ternary 2-bit weights, group-128 scales, dynamic int8 activation
quant) for Trainium2, tensor-parallel over 8 NeuronCores (shard N).

Math (per core, N-shard ns):
  s[m]   = 127 / clip(max_k |x[m,k]|, 1e-5)
  q[m,k] = round(x[m,k] * s[m])                      (integers in [-127,127])
  out[m,n] = (sum_k q[m,k] * w[n,k] * ws[n, k//128]) / s[m]   -> bf16

Device scheme ("stream-W" + int16 transposes):
- W is decoded host-side to full bf16 W[k,n] = (code-1)*ws (exact) and
  streamed from HBM; no on-device decode at all.
- Quantization is ONE activation pass: t1 = x*s + 2^23 (RNE rounding via the
  fp32 magic number).  t1's low 16 bits ARE q as int16 two's complement, so
  the PE transposes t1.bitcast(int16) stride-2 views directly
  (is_transpose=True, 1 cycle/row for 16-bit dtypes); the PSUM->SBUF evac
  converts int16 -> bf16.  The explicit "subtract 2^23" pass disappears.
- Mains: psm[mh][nh] += qT-slice.T @ W-tile over 64 k-blocks.
- Schedule: x first at full HBM bw (W gated on rowmax progress via dummy
  writes); resident W units (k-blocks 0..15) serve early mh0 mains (P1) and
  mh1 catch-up (P3, before P2 so both psm tiles finish together); units 4-15
  stream through a rotating pool for paired mains (P2).
- Dense fp32 dummy-matmul blocks tied to x-chunk arrivals keep the PE DVFS
  ramped through the head phase.
"""

import sys

import numpy as np

try:
    import concourse.bass as bass
except ImportError:  # fresh grading dir: fall back to the repo checkout
    sys.path.insert(0, "/opt/trn_rl_repo")
    import concourse.bass as bass

import ml_dtypes

import concourse.mybir as mybir
import concourse.tile as tile
from concourse import bacc, bass_utils
from concourse.masks import make_identity

FP32 = mybir.dt.float32
BF16 = mybir.dt.bfloat16
I16 = mybir.dt.int16
F16 = mybir.dt.float16
# 1.5 * 2^23: fp32 RNE rounds x*s to an integer AND every value in
# [1.5*2^23 - 128, 1.5*2^23 + 127] keeps exponent 150, so the low 16
# mantissa bits are exactly q in two's complement.
MAGIC = float(3 << 22)

M, N, K, GS = 256, 8192, 8192, 128
NCORES = 8


def build_nc(m=M, k=K, ns=N // NCORES):
    """One core's program: full m,k; n-shard of size ns."""
    mt = m // 128        # m partition-tiles (2)
    ck = k // 2048       # 2048-wide k-chunks for quant (4)
    kb = k // 128        # k-blocks / contraction tiles (64)
    npr = kb // 8        # transpose pairs: 8 k-blocks per PSUM bank (8)
    wu = kb // 4         # W DMA units of 4 k-blocks, 1MB each (16)
    n_res = min(4, wu)   # resident W units (k-blocks 0 .. 4*n_res-1)
    bres = 4 * n_res
    nsl = min(512, ns)   # matmul rhs free-dim slice (PSUM bank width)
    nh_n = ns // nsl

    nc = bacc.Bacc()
    x_d = nc.declare_dram_parameter("x", [m, k], FP32, isOutput=False)
    w_d = nc.declare_dram_parameter("wf", [k, ns], BF16, isOutput=False)
    out_d = nc.declare_dram_parameter("out", [m, ns], BF16, isOutput=True)

    x_r = x_d.rearrange("(T p) k -> T p k", p=128)            # [mt,128,k]
    w_r = w_d.rearrange("(u f p) n -> u p f n", f=4, p=128)   # [wu,128,4,ns]
    out_r = out_d.rearrange("(T p) n -> T p n", p=128)        # [mt,128,ns]

    with tile.TileContext(nc) as tc:
        with (
            tc.tile_pool(name="const", bufs=1) as constp,
            tc.tile_pool(name="stat", bufs=1) as statp,
            tc.tile_pool(name="qt", bufs=1) as qtp,
            tc.tile_pool(name="xp", bufs=8) as xp,
            tc.tile_pool(name="t1", bufs=4) as t1p,
            tc.tile_pool(name="wres", bufs=1) as wresp,
            tc.tile_pool(name="wstr", bufs=4) as wstrp,
            tc.tile_pool(name="ob", bufs=1) as obp,
            tc.tile_pool(name="psx", bufs=1, space="PSUM") as psxp,
            tc.tile_pool(name="pst", bufs=3, space="PSUM") as pstp,
            tc.tile_pool(name="psm", bufs=1, space="PSUM") as psmp,
        ):
            ident = constp.tile([128, 128], F16, tag="ident")
            make_identity(nc, ident)

            def warm(dep_tile, width, n_mm=1, name=""):
                """Dense fp32 dummy matmuls reading a landed tile: keep the
                PE DVFS/HAM ramped through the head phase."""
                for j in range(n_mm):
                    wp_ = psxp.tile([128, 512], FP32, tag="psx",
                                    name=f"wrm{name}{j}")
                    nc.tensor.matmul(
                        wp_[:, :width], dep_tile[:, :128],
                        dep_tile[:, :width],
                        start=True, stop=True,
                    )

            # t1 tiles (q in the low int16 of each fp32), per (mh, chunk)
            t1s = {}
            # transposed q pairs: [128, 8 kb-sub, 128 m] bf16 per (mh, pair)
            qtqp = {
                (mh, pr): qtp.tile([128, 1024], BF16, tag=f"qt{mh}_{pr}",
                                   name=f"qt{mh}_{pr}")
                for mh in range(mt) for pr in range(npr)
            }

            rpart = [statp.tile([128, ck], FP32, tag=f"rp{t}", name=f"rp{t}")
                     for t in range(mt)]
            rmax = [statp.tile([128, 1], FP32, tag=f"rm{t}", name=f"rm{t}")
                    for t in range(mt)]
            s_pp = [statp.tile([128, 1], FP32, tag=f"sp{t}", name=f"sp{t}")
                    for t in range(mt)]
            r1s = [statp.tile([128, 1], FP32, tag=f"rs{t}", name=f"rs{t}")
                   for t in range(mt)]

            psm = [
                [psmp.tile([128, nsl], FP32, tag=f"ps{mh}{nh}",
                           name=f"ps{mh}{nh}")
                 for nh in range(nh_n)]
                for mh in range(mt)
            ]

            wtiles = {}

            def load_w(u, pool, gate=None):
                wt = pool.tile([128, 4 * ns], BF16,
                               tag="wt" if pool is wstrp else f"wr{u}",
                               name=f"w{u}")
                if gate is not None:
                    # dummy write reading `gate`: delays this DMA until gate
                    # is produced (keeps W off the HBM while x streams)
                    nc.gpsimd.tensor_copy(wt[:1, :1], gate[:1, :1])
                wt3 = wt.rearrange("p (f n) -> p f n", f=4)
                nc.sync.dma_start(wt3[:], w_r[u])
                wtiles[u] = wt3

            def load_x(mh):
                xs = []
                for c in range(ck):
                    sl = slice(2048 * c, 2048 * (c + 1))
                    xc = xp.tile([128, 2048], FP32, tag="x", name=f"x{mh}{c}")
                    nc.sync.dma_start(xc[:], x_r[mh, :, sl])
                    xs.append(xc)
                return xs

            def rowmax_part(mh, c, xc):
                nc.vector.tensor_reduce(
                    rpart[mh][:, c : c + 1], xc[:],
                    axis=mybir.AxisListType.X, op=mybir.AluOpType.max,
                    apply_absolute_value=True,
                )

            def rowmax_fin(mh):
                nc.vector.tensor_reduce(
                    rmax[mh][:], rpart[mh][:],
                    axis=mybir.AxisListType.X, op=mybir.AluOpType.max,
                )
                nc.vector.tensor_scalar_max(rmax[mh][:], rmax[mh][:], 1e-5)
                nc.vector.reciprocal(s_pp[mh][:], rmax[mh][:])
                nc.vector.tensor_scalar_mul(s_pp[mh][:], s_pp[mh][:], 127.0)
                nc.vector.tensor_scalar_mul(r1s[mh][:], rmax[mh][:],
                                            1.0 / 127.0)

            def pass1(mh, c, xc):
                # t1 = x*s + 2^23: fp32 RNE puts q = round(x*s) in the low
                # mantissa bits; the int16 view of t1 IS q (two's complement).
                t1 = t1p.tile([128, 2048], FP32, tag="t1", name=f"t1_{mh}{c}")
                nc.scalar.activation(
                    t1[:], xc[:], mybir.ActivationFunctionType.Copy,
                    bias=MAGIC, scale=s_pp[mh][:],
                )
                t1s[(mh, c)] = t1

            def transpose_pair(mh, pr):
                """Transpose k-blocks 8pr..8pr+7 of m-tile mh into one PSUM
                bank via int16 is_transpose matmuls; return psT for evac."""
                psT = pstp.tile([128, 1024], F16, tag="psT")
                for j in range(8):
                    b = 8 * pr + j
                    c, jj = b // 16, b % 16
                    # fp16 bitcast view: the transpose is a raw bit move, the
                    # evac below reinterprets the bits as int16
                    qv = t1s[(mh, c)].bitcast(F16).rearrange(
                        "p (kk two) -> p kk two", two=2)[:, :, 0]
                    nc.tensor.transpose(
                        psT[:, 128 * j : 128 * (j + 1)],
                        qv[:, 128 * jj : 128 * (jj + 1)], ident[:],
                    )
                return psT

            def evac_pair(mh, pr, psT, eng):
                dst = qtqp[(mh, pr)][:]
                src_i16 = psT[:].bitcast(I16)
                if eng == 0:
                    nc.scalar.activation(
                        dst, src_i16, mybir.ActivationFunctionType.Copy
                    )
                else:
                    nc.vector.tensor_copy(dst, src_i16)

            started = [[False] * nh_n for _ in range(mt)]

            def mains(mh, b, stop=False):
                u = b // 4
                wt3 = wtiles[u]
                lhsT = qtqp[(mh, b // 8)].rearrange(
                    "p (f mm) -> p f mm", f=8)[:, b % 8, :]
                for nh in range(nh_n):
                    nc.tensor.matmul(
                        psm[mh][nh][:],
                        lhsT,
                        wt3[:, b % 4, nsl * nh : nsl * (nh + 1)],
                        start=not started[mh][nh], stop=stop,
                    )
                    started[mh][nh] = True

            def finalize(mh):
                # evac both psm tiles of this m-tile into one buffer, then a
                # single DMA; mh0 drives ACT+scalar queue, mh1 DVE+vector
                # queue so the two pipelines drain in parallel.
                ob = obp.tile([128, ns], BF16, tag=f"ob{mh}", name=f"ob{mh}")
                for nh in range(nh_n):
                    seg = ob[:, nsl * nh : nsl * (nh + 1)]
                    if mh == 0:
                        nc.scalar.activation(
                            seg, psm[mh][nh][:],
                            mybir.ActivationFunctionType.Copy,
                            scale=r1s[mh][:],
                        )
                    else:
                        nc.vector.tensor_scalar_mul(
                            seg, psm[mh][nh][:], r1s[mh][:]
                        )
                if mh == 0:
                    nc.scalar.dma_start(out_r[mh], ob[:])
                else:
                    nc.gpsimd.dma_start(out_r[mh], ob[:])

            # ---------------- emission schedule ----------------
            xs0 = load_x(0)
            xs1 = load_x(1)     # both m-tiles stream at full bandwidth
            # PE warms tied to x chunk arrivals (fp32, dense) bridge the head
            for c in range(ck):
                warm(xs0[c], 512, n_mm=2 if c < ck - 1 else 4, name=f"a{c}")
            warm(xs1[0], 512, n_mm=2, name="b0")
            for c in range(ck):
                rowmax_part(0, c, xs0[c])
            rowmax_fin(0)
            # W residents gated on the last mt0 partial (x keeps the HBM)
            for u in range(n_res):
                load_w(u, wresp, gate=rpart[0][:, ck - 1 : ck])
            for c in range(ck):
                pass1(0, c, xs0[c])
            # mt0 pairs 0-1: transpose + DVE evac feed P1; first mt1 rowmax
            # chunk slots between them on the DVE queue
            psTa = transpose_pair(0, 0)
            psTb = transpose_pair(0, 1)
            rowmax_part(1, 0, xs1[0])
            evac_pair(0, 0, psTa, eng=1)
            evac_pair(0, 1, psTb, eng=1)
            for c in range(1, ck):
                rowmax_part(1, c, xs1[c])
            rowmax_fin(1)
            # P1 (early mh0 mains on residents) interleaved with the rest of
            # the mt0 transposes; pairs 2-3 evac on ACT, 4+ on DVE (after the
            # mt1 rowmax chain in the DVE FIFO)
            for pr in range(2, npr):
                psT = transpose_pair(0, pr)
                evac_pair(0, pr, psT, eng=0 if pr < 4 else 1)
                lo, hi = 8 * (pr - 2), min(8 * (pr - 1), bres)
                for b in range(lo, hi):
                    mains(0, b, stop=(wu == n_res and b == kb - 1))
            for b in range(max(0, 8 * (npr - 2)), bres):
                mains(0, b, stop=(wu == n_res and b == kb - 1))
            for u in range(n_res, min(n_res + 4, wu)):
                load_w(u, wstrp, gate=rmax[1])
            # mt1 quant; transposes of pairs 0-1 feed P3 (mh1 catch-up on the
            # residents, before P2 so all psm tiles finish together)
            for c in range(ck):
                pass1(1, c, xs1[c])
            for pr in range(2):
                psT = transpose_pair(1, pr)
                evac_pair(1, pr, psT, eng=1)
            for b in range(0, bres):
                mains(1, b, stop=(wu == n_res and b == kb - 1))
            for pr in range(2, npr):
                psT = transpose_pair(1, pr)
                evac_pair(1, pr, psT, eng=1)
            for u in range(n_res + 4, wu):
                load_w(u, wstrp)
            # P2: paired mains on streamed W
            for b in range(bres, kb):
                mains(0, b, stop=(b == kb - 1))
                mains(1, b, stop=(b == kb - 1))
            finalize(0)
            finalize(1)
    nc.compile()
    return nc


def host_prep(input, weight_scale, weight, ns):
    """Shard + relayout inputs for each core: decode the packed 2-bit ternary
    codes and fold the per-(row, group) scale into a full bf16 W[k, n] per
    core (pure static-weight relayout), plus fp32 activation passthrough."""
    n, k4 = weight.shape
    k = k4 * 4
    x = np.ascontiguousarray(input, dtype=np.float32)
    w_bytes = weight.astype(np.uint8)                       # [N, K/4]
    codes = np.empty((n, k), dtype=np.int8)                 # [N, K] in {-1,0,1}
    for j in range(4):
        codes[:, j::4] = ((w_bytes >> (2 * j)) & 3).astype(np.int8) - 1
    ws2 = np.asarray(weight_scale, dtype=np.float32).reshape(n, -1)  # [N, K/GS]
    ws2_b = ws2.astype(ml_dtypes.bfloat16)
    # W[n, k] = codes * ws (exact in bf16: +-1 * bf16 scale)
    wf = codes.astype(np.float32) * ws2_b.astype(np.float32).repeat(GS, axis=1)
    wf = wf.astype(ml_dtypes.bfloat16)
    in_maps = []
    for c in range(n // ns):
        sl = slice(c * ns, (c + 1) * ns)
        wf_c = np.ascontiguousarray(wf[sl].T)               # [K, ns] bf16
        in_maps.append({"x": x, "wf": wf_c})
    return in_maps


_NC_CACHE = {}


def _get_nc(m, k, ns):
    key = (m, k, ns)
    if key not in _NC_CACHE:
        _NC_CACHE[key] = build_nc(m, k, ns)
    return _NC_CACHE[key]


def kernel(input, weight_scale, weight, group_size=GS, trace=False):
    m, k = input.shape
    n = weight.shape[0]
    ns = n // NCORES
    nc = _get_nc(m, k, ns)
    in_maps = host_prep(input, weight_scale, weight, ns)
    res = bass_utils.run_bass_kernel_spmd(
        nc, in_maps, core_ids=list(range(NCORES)), trace=trace
    )
    out = np.concatenate([r["out"] for r in res.results], axis=1)
    if trace:
        return out, res
    return out


if __name__ == "__main__":
    # small-config CoreSim check
    from concourse.bass_interp import CoreSim

    rng = np.random.default_rng(0)
    m, k, ns = 256, 2048, 256
    x = rng.standard_normal((m, k), dtype=np.float32)
    w_tern = rng.integers(-1, 2, size=(ns, k)).astype(np.int32)
    codes = (w_tern + 1).reshape(ns, k // 4, 4)
    packed = (
        codes[..., 0] | (codes[..., 1] << 2) | (codes[..., 2] << 4)
        | (codes[..., 3] << 6)
    ).astype(np.int32)
    ws = rng.uniform(0.001, 0.02, size=(ns, k // GS, 1)).astype(np.float32)

    # numpy reference
    s = 127.0 / np.clip(np.abs(x).max(axis=-1, keepdims=True), 1e-5, None)
    q = np.clip(np.round(x * s), -128, 127)
    wf = w_tern.astype(np.float32) * np.repeat(ws.reshape(ns, -1), GS, axis=1)
    ref = ((q @ wf.T) / s).astype(ml_dtypes.bfloat16).astype(np.float32)

    nc = build_nc(m, k, ns)
    im = host_prep(x, ws, packed, ns)[0]
    sim = CoreSim(nc)
    for kk, v in im.items():
        sim.tensor(kk)[:] = v
    sim.simulate()
    got = np.asarray(sim.tensor("out")).astype(np.float32)
    err = np.abs(got - ref).max() / (np.abs(ref).max() + 1e-9)
    print("rel err (absmax):", err)
    rms = np.sqrt(((got - ref) ** 2).mean()) / (np.sqrt((ref**2).mean()) + 1e-9)
    print("rel err (rms):", rms)


# revision 14
# speedup vs baseline: 1.1480x; 1.1480x over previous
"""BitLinear (ternary 2-bit weights, group-128 scales, dynamic int8 activation
quant) for Trainium2, tensor-parallel over 8 NeuronCores (shard N).

Math (per core, N-shard ns):
  s[m]   = 127 / clip(max_k |x[m,k]|, 1e-5)
  q[m,k] = round(x[m,k] * s[m])                      (integers in [-127,127])
  out[m,n] = (sum_k q[m,k] * w[n,k] * ws[n, k//128]) / s[m]   -> bf16

Device scheme ("stream-W" + int16 transposes):
- W is decoded host-side to full bf16 W[k,n] = (code-1)*ws (exact) and
  streamed from HBM; no on-device decode at all.
- Quantization is ONE activation pass: t1 = x*s + 2^23 (RNE rounding via the
  fp32 magic number).  t1's low 16 bits ARE q as int16 two's complement, so
  the PE transposes t1.bitcast(int16) stride-2 views directly
  (is_transpose=True, 1 cycle/row for 16-bit dtypes); the PSUM->SBUF evac
  converts int16 -> bf16.  The explicit "subtract 2^23" pass disappears.
- Mains: psm[mh][nh] += qT-slice.T @ W-tile over 64 k-blocks.
- Schedule: x first at full HBM bw (W gated on rowmax progress via dummy
  writes); resident W units (k-blocks 0..15) serve early mh0 mains (P1) and
  mh1 catch-up (P3, before P2 so both psm tiles finish together); units 4-15
  stream through a rotating pool for paired mains (P2).
- Dense fp32 dummy-matmul blocks tied to x-chunk arrivals keep the PE DVFS
  ramped through the head phase.
"""

import sys

import numpy as np

try:
    import concourse.bass as bass
except ImportError:  # fresh grading dir: fall back to the repo checkout
    sys.path.insert(0, "/opt/trn_rl_repo")
    import concourse.bass as bass

import ml_dtypes

import concourse.mybir as mybir
import concourse.tile as tile
from concourse import bacc, bass_utils
from concourse.masks import make_identity

FP32 = mybir.dt.float32
BF16 = mybir.dt.bfloat16
I16 = mybir.dt.int16
F16 = mybir.dt.float16
# 1.5 * 2^23: fp32 RNE rounds x*s to an integer AND every value in
# [1.5*2^23 - 128, 1.5*2^23 + 127] keeps exponent 150, so the low 16
# mantissa bits are exactly q in two's complement.
MAGIC = float(3 << 22)

M, N, K, GS = 256, 8192, 8192, 128
NCORES = 8


def build_nc(m=M, k=K, ns=N // NCORES):
    """One core's program: full m,k; n-shard of size ns."""
    mt = m // 128        # m partition-tiles (2)
    ck = k // 2048       # 2048-wide k-chunks for quant (4)
    kb = k // 128        # k-blocks / contraction tiles (64)
    npr = kb // 8        # transpose pairs: 8 k-blocks per PSUM bank (8)
    wu = kb // 4         # W DMA units of 4 k-blocks, 1MB each (16)
    n_res = min(4, wu)   # resident W units (k-blocks 0 .. 4*n_res-1)
    bres = 4 * n_res
    nsl = min(512, ns)   # matmul rhs free-dim slice (PSUM bank width)
    nh_n = ns // nsl

    nc = bacc.Bacc()
    x_d = nc.declare_dram_parameter("x", [m, k], FP32, isOutput=False)
    w_d = nc.declare_dram_parameter("wf", [k, ns], BF16, isOutput=False)
    out_d = nc.declare_dram_parameter("out", [m, ns], BF16, isOutput=True)

    x_r = x_d.rearrange("(T p) k -> T p k", p=128)            # [mt,128,k]
    w_r = w_d.rearrange("(u f p) n -> u p f n", f=4, p=128)   # [wu,128,4,ns]
    out_r = out_d.rearrange("(T p) n -> T p n", p=128)        # [mt,128,ns]

    with tile.TileContext(nc) as tc:
        with (
            tc.tile_pool(name="const", bufs=1) as constp,
            tc.tile_pool(name="stat", bufs=1) as statp,
            tc.tile_pool(name="qt", bufs=1) as qtp,
            tc.tile_pool(name="xp", bufs=8) as xp,
            tc.tile_pool(name="t1", bufs=4) as t1p,
            tc.tile_pool(name="wres", bufs=1) as wresp,
            tc.tile_pool(name="wstr", bufs=4) as wstrp,
            tc.tile_pool(name="ob", bufs=1) as obp,
            tc.tile_pool(name="psx", bufs=1, space="PSUM") as psxp,
            tc.tile_pool(name="pst", bufs=3, space="PSUM") as pstp,
            tc.tile_pool(name="psm", bufs=1, space="PSUM") as psmp,
        ):
            ident = constp.tile([128, 128], F16, tag="ident")
            make_identity(nc, ident)

            def warm(dep_tile, width, n_mm=1, name=""):
                """Dense fp32 dummy matmuls reading a landed tile: keep the
                PE DVFS/HAM ramped through the head phase."""
                for j in range(n_mm):
                    wp_ = psxp.tile([128, 512], FP32, tag="psx",
                                    name=f"wrm{name}{j}")
                    nc.tensor.matmul(
                        wp_[:, :width], dep_tile[:, :128],
                        dep_tile[:, :width],
                        start=True, stop=True,
                    )

            # t1 tiles (q in the low int16 of each fp32), per (mh, chunk)
            t1s = {}
            # transposed q pairs: [128, 8 kb-sub, 128 m] bf16 per (mh, pair)
            qtqp = {
                (mh, pr): qtp.tile([128, 1024], BF16, tag=f"qt{mh}_{pr}",
                                   name=f"qt{mh}_{pr}")
                for mh in range(mt) for pr in range(npr)
            }

            rpart = [statp.tile([128, ck], FP32, tag=f"rp{t}", name=f"rp{t}")
                     for t in range(mt)]
            rmax = [statp.tile([128, 1], FP32, tag=f"rm{t}", name=f"rm{t}")
                    for t in range(mt)]
            s_pp = [statp.tile([128, 1], FP32, tag=f"sp{t}", name=f"sp{t}")
                    for t in range(mt)]
            r1s = [statp.tile([128, 1], FP32, tag=f"rs{t}", name=f"rs{t}")
                   for t in range(mt)]

            psm = [
                [psmp.tile([128, nsl], FP32, tag=f"ps{mh}{nh}",
                           name=f"ps{mh}{nh}")
                 for nh in range(nh_n)]
                for mh in range(mt)
            ]

            wtiles = {}

            def load_w(u, pool, gate=None):
                wt = pool.tile([128, 4 * ns], BF16,
                               tag="wt" if pool is wstrp else f"wr{u}",
                               name=f"w{u}")
                if gate is not None:
                    # dummy write reading `gate`: delays this DMA until gate
                    # is produced (keeps W off the HBM while x streams)
                    nc.gpsimd.tensor_copy(wt[:1, :1], gate[:1, :1])
                wt3 = wt.rearrange("p (f n) -> p f n", f=4)
                nc.sync.dma_start(wt3[:], w_r[u])
                wtiles[u] = wt3

            def load_x(mh):
                xs = []
                for c in range(ck):
                    sl = slice(2048 * c, 2048 * (c + 1))
                    xc = xp.tile([128, 2048], FP32, tag="x", name=f"x{mh}{c}")
                    nc.sync.dma_start(xc[:], x_r[mh, :, sl])
                    xs.append(xc)
                return xs

            def rowmax_part(mh, c, xc):
                nc.vector.tensor_reduce(
                    rpart[mh][:, c : c + 1], xc[:],
                    axis=mybir.AxisListType.X, op=mybir.AluOpType.max,
                    apply_absolute_value=True,
                )

            def rowmax_fin(mh):
                nc.vector.tensor_reduce(
                    rmax[mh][:], rpart[mh][:],
                    axis=mybir.AxisListType.X, op=mybir.AluOpType.max,
                )
                nc.vector.tensor_scalar_max(rmax[mh][:], rmax[mh][:], 1e-5)
                nc.vector.reciprocal(s_pp[mh][:], rmax[mh][:])
                nc.vector.tensor_scalar_mul(s_pp[mh][:], s_pp[mh][:], 127.0)
                nc.vector.tensor_scalar_mul(r1s[mh][:], rmax[mh][:],
                                            1.0 / 127.0)

            def pass1(mh, c, xc):
                # t1 = x*s + 2^23: fp32 RNE puts q = round(x*s) in the low
                # mantissa bits; the int16 view of t1 IS q (two's complement).
                t1 = t1p.tile([128, 2048], FP32, tag="t1", name=f"t1_{mh}{c}")
                nc.scalar.activation(
                    t1[:], xc[:], mybir.ActivationFunctionType.Copy,
                    bias=MAGIC, scale=s_pp[mh][:],
                )
                t1s[(mh, c)] = t1

            def transpose_pair(mh, pr):
                """Transpose k-blocks 8pr..8pr+7 of m-tile mh into one PSUM
                bank via int16 is_transpose matmuls; return psT for evac."""
                psT = pstp.tile([128, 1024], F16, tag="psT")
                for j in range(8):
                    b = 8 * pr + j
                    c, jj = b // 16, b % 16
                    # fp16 bitcast view: the transpose is a raw bit move, the
                    # evac below reinterprets the bits as int16
                    qv = t1s[(mh, c)].bitcast(F16).rearrange(
                        "p (kk two) -> p kk two", two=2)[:, :, 0]
                    nc.tensor.transpose(
                        psT[:, 128 * j : 128 * (j + 1)],
                        qv[:, 128 * jj : 128 * (jj + 1)], ident[:],
                    )
                return psT

            def evac_pair(mh, pr, psT, eng):
                dst = qtqp[(mh, pr)][:]
                src_i16 = psT[:].bitcast(I16)
                if eng == 0:
                    nc.scalar.activation(
                        dst, src_i16, mybir.ActivationFunctionType.Copy
                    )
                else:
                    nc.vector.tensor_copy(dst, src_i16)

            started = [[False] * nh_n for _ in range(mt)]

            def mains(mh, b, stop=False):
                u = b // 4
                wt3 = wtiles[u]
                lhsT = qtqp[(mh, b // 8)].rearrange(
                    "p (f mm) -> p f mm", f=8)[:, b % 8, :]
                for nh in range(nh_n):
                    nc.tensor.matmul(
                        psm[mh][nh][:],
                        lhsT,
                        wt3[:, b % 4, nsl * nh : nsl * (nh + 1)],
                        start=not started[mh][nh], stop=stop,
                    )
                    started[mh][nh] = True

            def finalize(mh):
                # evac both psm tiles into one buffer (ACT), single DMA
                ob = obp.tile([128, ns], BF16, tag=f"ob{mh}", name=f"ob{mh}")
                for nh in range(nh_n):
                    nc.scalar.activation(
                        ob[:, nsl * nh : nsl * (nh + 1)], psm[mh][nh][:],
                        mybir.ActivationFunctionType.Copy, scale=r1s[mh][:],
                    )
                nc.scalar.dma_start(out_r[mh], ob[:])

            # ---------------- emission schedule ----------------
            xs0 = load_x(0)
            xs1 = load_x(1)     # both m-tiles stream at full bandwidth
            # PE warms tied to x chunk arrivals (fp32, dense) bridge the head
            for c in range(ck):
                warm(xs0[c], 512, n_mm=2 if c < ck - 1 else 4, name=f"a{c}")
            warm(xs1[0], 512, n_mm=2, name="b0")
            for c in range(ck):
                rowmax_part(0, c, xs0[c])
            rowmax_fin(0)
            # W residents gated on the last mt0 partial (x keeps the HBM)
            for u in range(n_res):
                load_w(u, wresp, gate=rpart[0][:, ck - 1 : ck])
            for c in range(ck):
                pass1(0, c, xs0[c])
            # mt0 pairs 0-1: transpose + DVE evac feed P1; first mt1 rowmax
            # chunk slots between them on the DVE queue
            psTa = transpose_pair(0, 0)
            psTb = transpose_pair(0, 1)
            rowmax_part(1, 0, xs1[0])
            evac_pair(0, 0, psTa, eng=1)
            evac_pair(0, 1, psTb, eng=1)
            for c in range(1, ck):
                rowmax_part(1, c, xs1[c])
            rowmax_fin(1)
            # P1 (early mh0 mains on residents) interleaved with the rest of
            # the mt0 transposes; pairs 2-3 evac on ACT, 4+ on DVE (after the
            # mt1 rowmax chain in the DVE FIFO)
            for pr in range(2, npr):
                psT = transpose_pair(0, pr)
                evac_pair(0, pr, psT, eng=0 if pr < 4 else 1)
                lo, hi = 8 * (pr - 2), min(8 * (pr - 1), bres)
                for b in range(lo, hi):
                    mains(0, b, stop=(wu == n_res and b == kb - 1))
            for b in range(max(0, 8 * (npr - 2)), bres):
                mains(0, b, stop=(wu == n_res and b == kb - 1))
            for u in range(n_res, min(n_res + 4, wu)):
                load_w(u, wstrp, gate=rmax[1])
            # mt1 quant; transposes of pairs 0-1 feed P3 (mh1 catch-up on the
            # residents, before P2 so all psm tiles finish together)
            for c in range(ck):
                pass1(1, c, xs1[c])
            for pr in range(2):
                psT = transpose_pair(1, pr)
                evac_pair(1, pr, psT, eng=1)
            for b in range(0, bres):
                mains(1, b, stop=(wu == n_res and b == kb - 1))
            for pr in range(2, npr):
                psT = transpose_pair(1, pr)
                evac_pair(1, pr, psT, eng=1)
            for u in range(n_res + 4, wu):
                load_w(u, wstrp)
            # P2: paired mains on streamed W
            for b in range(bres, kb):
                mains(0, b, stop=(b == kb - 1))
                mains(1, b, stop=(b == kb - 1))
            finalize(0)
            finalize(1)
    nc.compile()
    return nc


def host_prep(input, weight_scale, weight, ns):
    """Shard + relayout inputs for each core: decode the packed 2-bit ternary
    codes and fold the per-(row, group) scale into a full bf16 W[k, n] per
    core (pure static-weight relayout), plus fp32 activation passthrough."""
    n, k4 = weight.shape
    k = k4 * 4
    x = np.ascontiguousarray(input, dtype=np.float32)
    w_bytes = weight.astype(np.uint8)                       # [N, K/4]
    codes = np.empty((n, k), dtype=np.int8)                 # [N, K] in {-1,0,1}
    for j in range(4):
        codes[:, j::4] = ((w_bytes >> (2 * j)) & 3).astype(np.int8) - 1
    ws2 = np.asarray(weight_scale, dtype=np.float32).reshape(n, -1)  # [N, K/GS]
    ws2_b = ws2.astype(ml_dtypes.bfloat16)
    # W[n, k] = codes * ws (exact in bf16: +-1 * bf16 scale)
    wf = codes.astype(np.float32) * ws2_b.astype(np.float32).repeat(GS, axis=1)
    wf = wf.astype(ml_dtypes.bfloat16)
    in_maps = []
    for c in range(n // ns):
        sl = slice(c * ns, (c + 1) * ns)
        wf_c = np.ascontiguousarray(wf[sl].T)               # [K, ns] bf16
        in_maps.append({"x": x, "wf": wf_c})
    return in_maps


_NC_CACHE = {}


def _get_nc(m, k, ns):
    key = (m, k, ns)
    if key not in _NC_CACHE:
        _NC_CACHE[key] = build_nc(m, k, ns)
    return _NC_CACHE[key]


def kernel(input, weight_scale, weight, group_size=GS, trace=False):
    m, k = input.shape
    n = weight.shape[0]
    ns = n // NCORES
    nc = _get_nc(m, k, ns)
    in_maps = host_prep(input, weight_scale, weight, ns)
    res = bass_utils.run_bass_kernel_spmd(
        nc, in_maps, core_ids=list(range(NCORES)), trace=trace
    )
    out = np.concatenate([r["out"] for r in res.results], axis=1)
    if trace:
        return out, res
    return out


if __name__ == "__main__":
    # small-config CoreSim check
    from concourse.bass_interp import CoreSim

    rng = np.random.default_rng(0)
    m, k, ns = 256, 2048, 256
    x = rng.standard_normal((m, k), dtype=np.float32)
    w_tern = rng.integers(-1, 2, size=(ns, k)).astype(np.int32)
    codes = (w_tern + 1).reshape(ns, k // 4, 4)
    packed = (
        codes[..., 0] | (codes[..., 1] << 2) | (codes[..., 2] << 4)
        | (codes[..., 3] << 6)
    ).astype(np.int32)
    ws = rng.uniform(0.001, 0.02, size=(ns, k // GS, 1)).astype(np.float32)

    # numpy reference
    s = 127.0 / np.clip(np.abs(x).max(axis=-1, keepdims=True), 1e-5, None)
    q = np.clip(np.round(x * s), -128, 127)
    wf = w_tern.astype(np.float32) * np.repeat(ws.reshape(ns, -1), GS, axis=1)
    ref = ((q @ wf.T) / s).astype(ml_dtypes.bfloat16).astype(np.float32)

    nc = build_nc(m, k, ns)
    im = host_prep(x, ws, packed, ns)[0]
    sim = CoreSim(nc)
    for kk, v in im.items():
        sim.tensor(kk)[:] = v
    sim.simulate()
    got = np.asarray(sim.tensor("out")).astype(np.float32)
    err = np.abs(got - ref).max() / (np.abs(ref).max() + 1e-9)
    print("rel err (absmax):", err)
    rms = np.sqrt(((got - ref) ** 2).mean()) / (np.sqrt((ref**2).mean()) + 1e-9)
    print("rel err (rms):", rms)


# revision 15
# speedup vs baseline: 1.1820x; 1.0296x over previous
"""BitLinear (ternary 2-bit weights, group-128 scales, dynamic int8 activation
quant) for Trainium2, tensor-parallel over 8 NeuronCores (shard N).

Math (per core, N-shard ns):
  s[m]   = 127 / clip(max_k |x[m,k]|, 1e-5)
  q[m,k] = round(x[m,k] * s[m])                      (integers in [-127,127])
  out[m,n] = (sum_k q[m,k] * w[n,k] * ws[n, k//128]) / s[m]   -> bf16

Device scheme ("stream-W" + int16 transposes):
- W is decoded host-side to full bf16 W[k,n] = (code-1)*ws (exact) and
  streamed from HBM; no on-device decode at all.
- Quantization is ONE activation pass: t1 = x*s + 2^23 (RNE rounding via the
  fp32 magic number).  t1's low 16 bits ARE q as int16 two's complement, so
  the PE transposes t1.bitcast(int16) stride-2 views directly
  (is_transpose=True, 1 cycle/row for 16-bit dtypes); the PSUM->SBUF evac
  converts int16 -> bf16.  The explicit "subtract 2^23" pass disappears.
- Mains: psm[mh][nh] += qT-slice.T @ W-tile over 64 k-blocks.
- Schedule: x first at full HBM bw (W gated on rowmax progress via dummy
  writes); resident W units (k-blocks 0..15) serve early mh0 mains (P1) and
  mh1 catch-up (P3, before P2 so both psm tiles finish together); units 4-15
  stream through a rotating pool for paired mains (P2).
- Dense fp32 dummy-matmul blocks tied to x-chunk arrivals keep the PE DVFS
  ramped through the head phase.
"""

import sys

import numpy as np

try:
    import concourse.bass as bass
except ImportError:  # fresh grading dir: fall back to the repo checkout
    sys.path.insert(0, "/opt/trn_rl_repo")
    import concourse.bass as bass

import ml_dtypes

import concourse.mybir as mybir
import concourse.tile as tile
from concourse import bacc, bass_utils
from concourse.masks import make_identity

FP32 = mybir.dt.float32
BF16 = mybir.dt.bfloat16
I16 = mybir.dt.int16
F16 = mybir.dt.float16
# 1.5 * 2^23: fp32 RNE rounds x*s to an integer AND every value in
# [1.5*2^23 - 128, 1.5*2^23 + 127] keeps exponent 150, so the low 16
# mantissa bits are exactly q in two's complement.
MAGIC = float(3 << 22)

M, N, K, GS = 256, 8192, 8192, 128
NCORES = 8


def build_nc(m=M, k=K, ns=N // NCORES):
    """One core's program: full m,k; n-shard of size ns."""
    mt = m // 128        # m partition-tiles (2)
    ck = k // 2048       # 2048-wide k-chunks for quant (4)
    kb = k // 128        # k-blocks / contraction tiles (64)
    npr = kb // 8        # transpose pairs: 8 k-blocks per PSUM bank (8)
    wu = kb // 4         # W DMA units of 4 k-blocks, 1MB each (16)
    n_res = min(4, wu)   # resident W units (k-blocks 0 .. 4*n_res-1)
    bres = 4 * n_res
    nsl = min(512, ns)   # matmul rhs free-dim slice (PSUM bank width)
    nh_n = ns // nsl

    nc = bacc.Bacc()
    x_d = nc.declare_dram_parameter("x", [m, k], FP32, isOutput=False)
    w_d = nc.declare_dram_parameter("wf", [k, ns], BF16, isOutput=False)
    out_d = nc.declare_dram_parameter("out", [m, ns], BF16, isOutput=True)

    x_r = x_d.rearrange("(T p) k -> T p k", p=128)            # [mt,128,k]
    w_r = w_d.rearrange("(u f p) n -> u p f n", f=4, p=128)   # [wu,128,4,ns]
    out_r = out_d.rearrange("(T p) n -> T p n", p=128)        # [mt,128,ns]

    with tile.TileContext(nc) as tc:
        with (
            tc.tile_pool(name="const", bufs=1) as constp,
            tc.tile_pool(name="stat", bufs=1) as statp,
            tc.tile_pool(name="qt", bufs=1) as qtp,
            tc.tile_pool(name="xp", bufs=8) as xp,
            tc.tile_pool(name="t1", bufs=4) as t1p,
            tc.tile_pool(name="wres", bufs=1) as wresp,
            tc.tile_pool(name="wstr", bufs=4) as wstrp,
            tc.tile_pool(name="ob", bufs=1) as obp,
            tc.tile_pool(name="psx", bufs=1, space="PSUM") as psxp,
            tc.tile_pool(name="pst", bufs=3, space="PSUM") as pstp,
            tc.tile_pool(name="psm", bufs=1, space="PSUM") as psmp,
        ):
            ident = constp.tile([128, 128], F16, tag="ident")
            make_identity(nc, ident)

            def warm(dep_tile, width, n_mm=1, name=""):
                """Dense fp32 dummy matmuls reading a landed tile: keep the
                PE DVFS/HAM ramped through the head phase."""
                for j in range(n_mm):
                    wp_ = psxp.tile([128, 512], FP32, tag="psx",
                                    name=f"wrm{name}{j}")
                    nc.tensor.matmul(
                        wp_[:, :width], dep_tile[:, :128],
                        dep_tile[:, :width],
                        start=True, stop=True,
                    )

            # t1 tiles (q in the low int16 of each fp32), per (mh, chunk)
            t1s = {}
            # transposed q pairs: [128, 8 kb-sub, 128 m] bf16 per (mh, pair)
            qtqp = {
                (mh, pr): qtp.tile([128, 1024], BF16, tag=f"qt{mh}_{pr}",
                                   name=f"qt{mh}_{pr}")
                for mh in range(mt) for pr in range(npr)
            }

            rpart = [statp.tile([128, ck], FP32, tag=f"rp{t}", name=f"rp{t}")
                     for t in range(mt)]
            rmax = [statp.tile([128, 1], FP32, tag=f"rm{t}", name=f"rm{t}")
                    for t in range(mt)]
            s_pp = [statp.tile([128, 1], FP32, tag=f"sp{t}", name=f"sp{t}")
                    for t in range(mt)]
            r1s = [statp.tile([128, 1], FP32, tag=f"rs{t}", name=f"rs{t}")
                   for t in range(mt)]

            psm = [
                [psmp.tile([128, nsl], FP32, tag=f"ps{mh}{nh}",
                           name=f"ps{mh}{nh}")
                 for nh in range(nh_n)]
                for mh in range(mt)
            ]

            wtiles = {}

            def load_w(u, pool, gate=None):
                wt = pool.tile([128, 4 * ns], BF16,
                               tag="wt" if pool is wstrp else f"wr{u}",
                               name=f"w{u}")
                if gate is not None:
                    # dummy write reading `gate`: delays this DMA until gate
                    # is produced (keeps W off the HBM while x streams)
                    nc.gpsimd.tensor_copy(wt[:1, :1], gate[:1, :1])
                wt3 = wt.rearrange("p (f n) -> p f n", f=4)
                nc.sync.dma_start(wt3[:], w_r[u])
                wtiles[u] = wt3

            def load_x(mh):
                xs = []
                for c in range(ck):
                    sl = slice(2048 * c, 2048 * (c + 1))
                    xc = xp.tile([128, 2048], FP32, tag="x", name=f"x{mh}{c}")
                    nc.sync.dma_start(xc[:], x_r[mh, :, sl])
                    xs.append(xc)
                return xs

            def rowmax_part(mh, c, xc):
                nc.vector.tensor_reduce(
                    rpart[mh][:, c : c + 1], xc[:],
                    axis=mybir.AxisListType.X, op=mybir.AluOpType.max,
                    apply_absolute_value=True,
                )

            def rowmax_fin(mh):
                nc.vector.tensor_reduce(
                    rmax[mh][:], rpart[mh][:],
                    axis=mybir.AxisListType.X, op=mybir.AluOpType.max,
                )
                nc.vector.tensor_scalar_max(rmax[mh][:], rmax[mh][:], 1e-5)
                nc.vector.reciprocal(s_pp[mh][:], rmax[mh][:])
                nc.vector.tensor_scalar_mul(s_pp[mh][:], s_pp[mh][:], 127.0)
                nc.vector.tensor_scalar_mul(r1s[mh][:], rmax[mh][:],
                                            1.0 / 127.0)

            def pass1(mh, c, xc):
                # t1 = x*s + 2^23: fp32 RNE puts q = round(x*s) in the low
                # mantissa bits; the int16 view of t1 IS q (two's complement).
                t1 = t1p.tile([128, 2048], FP32, tag="t1", name=f"t1_{mh}{c}")
                nc.scalar.activation(
                    t1[:], xc[:], mybir.ActivationFunctionType.Copy,
                    bias=MAGIC, scale=s_pp[mh][:],
                )
                t1s[(mh, c)] = t1

            def transpose_pair(mh, pr):
                """Transpose k-blocks 8pr..8pr+7 of m-tile mh into one PSUM
                bank via int16 is_transpose matmuls; return psT for evac."""
                psT = pstp.tile([128, 1024], F16, tag="psT")
                for j in range(8):
                    b = 8 * pr + j
                    c, jj = b // 16, b % 16
                    # fp16 bitcast view: the transpose is a raw bit move, the
                    # evac below reinterprets the bits as int16
                    qv = t1s[(mh, c)].bitcast(F16).rearrange(
                        "p (kk two) -> p kk two", two=2)[:, :, 0]
                    nc.tensor.transpose(
                        psT[:, 128 * j : 128 * (j + 1)],
                        qv[:, 128 * jj : 128 * (jj + 1)], ident[:],
                    )
                return psT

            def evac_pair(mh, pr, psT, eng):
                dst = qtqp[(mh, pr)][:]
                src_i16 = psT[:].bitcast(I16)
                if eng == 0:
                    nc.scalar.activation(
                        dst, src_i16, mybir.ActivationFunctionType.Copy
                    )
                else:
                    nc.vector.tensor_copy(dst, src_i16)

            started = [[False] * nh_n for _ in range(mt)]

            def mains(mh, b, stop=False):
                u = b // 4
                wt3 = wtiles[u]
                lhsT = qtqp[(mh, b // 8)].rearrange(
                    "p (f mm) -> p f mm", f=8)[:, b % 8, :]
                for nh in range(nh_n):
                    nc.tensor.matmul(
                        psm[mh][nh][:],
                        lhsT,
                        wt3[:, b % 4, nsl * nh : nsl * (nh + 1)],
                        start=not started[mh][nh], stop=stop,
                    )
                    started[mh][nh] = True

            def finalize(mh):
                # evac both psm tiles into one buffer (ACT), single DMA
                ob = obp.tile([128, ns], BF16, tag=f"ob{mh}", name=f"ob{mh}")
                for nh in range(nh_n):
                    nc.scalar.activation(
                        ob[:, nsl * nh : nsl * (nh + 1)], psm[mh][nh][:],
                        mybir.ActivationFunctionType.Copy, scale=r1s[mh][:],
                    )
                nc.scalar.dma_start(out_r[mh], ob[:])

            # ---------------- emission schedule ----------------
            xs0 = load_x(0)
            xs1 = load_x(1)     # both m-tiles stream at full bandwidth
            # PE warms tied to x chunk arrivals (fp32, dense) bridge the head
            for c in range(ck):
                warm(xs0[c], 512, n_mm=2 if c < ck - 1 else 4, name=f"a{c}")
            warm(xs1[0], 512, n_mm=2, name="b0")
            for c in range(ck):
                rowmax_part(0, c, xs0[c])
            rowmax_fin(0)
            # W residents gated so both x m-tiles keep HBM priority: u0 on
            # the last mt0 partial, the rest on mt1 rowmax progress
            for u in range(n_res):
                gate = (rpart[0][:, ck - 1 : ck] if u == 0
                        else rpart[1][:, min(1, ck - 1) : min(1, ck - 1) + 1])
                load_w(u, wresp, gate=gate)
            for c in range(ck):
                pass1(0, c, xs0[c])
            # mt0 pairs 0-1: transpose + DVE evac feed P1; first mt1 rowmax
            # chunk slots between them on the DVE queue
            psTa = transpose_pair(0, 0)
            psTb = transpose_pair(0, 1)
            rowmax_part(1, 0, xs1[0])
            evac_pair(0, 0, psTa, eng=1)
            evac_pair(0, 1, psTb, eng=1)
            for c in range(1, ck):
                rowmax_part(1, c, xs1[c])
            rowmax_fin(1)
            # P1 (early mh0 mains on residents) interleaved with the rest of
            # the mt0 transposes; pairs 2-3 evac on ACT, 4+ on DVE (after the
            # mt1 rowmax chain in the DVE FIFO)
            for pr in range(2, npr):
                psT = transpose_pair(0, pr)
                evac_pair(0, pr, psT, eng=0 if pr < 4 else 1)
                lo, hi = 8 * (pr - 2), min(8 * (pr - 1), bres)
                for b in range(lo, hi):
                    mains(0, b, stop=(wu == n_res and b == kb - 1))
            for b in range(max(0, 8 * (npr - 2)), bres):
                mains(0, b, stop=(wu == n_res and b == kb - 1))
            for u in range(n_res, min(n_res + 4, wu)):
                load_w(u, wstrp, gate=rmax[1])
            warm(xs1[ck - 1], 512, n_mm=2, name="b3")
            # mt1 quant; transposes of pairs 0-1 feed P3 (mh1 catch-up on the
            # residents, before P2 so all psm tiles finish together)
            for c in range(ck):
                pass1(1, c, xs1[c])
            for pr in range(2):
                psT = transpose_pair(1, pr)
                evac_pair(1, pr, psT, eng=1)
            for b in range(0, bres):
                mains(1, b, stop=(wu == n_res and b == kb - 1))
            for pr in range(2, npr):
                psT = transpose_pair(1, pr)
                evac_pair(1, pr, psT, eng=1)
            for u in range(n_res + 4, wu):
                load_w(u, wstrp)
            # P2: paired mains on streamed W
            for b in range(bres, kb):
                mains(0, b, stop=(b == kb - 1))
                mains(1, b, stop=(b == kb - 1))
            finalize(0)
            finalize(1)
    nc.compile()
    return nc


def host_prep(input, weight_scale, weight, ns):
    """Shard + relayout inputs for each core: decode the packed 2-bit ternary
    codes and fold the per-(row, group) scale into a full bf16 W[k, n] per
    core (pure static-weight relayout), plus fp32 activation passthrough."""
    n, k4 = weight.shape
    k = k4 * 4
    x = np.ascontiguousarray(input, dtype=np.float32)
    w_bytes = weight.astype(np.uint8)                       # [N, K/4]
    codes = np.empty((n, k), dtype=np.int8)                 # [N, K] in {-1,0,1}
    for j in range(4):
        codes[:, j::4] = ((w_bytes >> (2 * j)) & 3).astype(np.int8) - 1
    ws2 = np.asarray(weight_scale, dtype=np.float32).reshape(n, -1)  # [N, K/GS]
    ws2_b = ws2.astype(ml_dtypes.bfloat16)
    # W[n, k] = codes * ws (exact in bf16: +-1 * bf16 scale)
    wf = codes.astype(np.float32) * ws2_b.astype(np.float32).repeat(GS, axis=1)
    wf = wf.astype(ml_dtypes.bfloat16)
    in_maps = []
    for c in range(n // ns):
        sl = slice(c * ns, (c + 1) * ns)
        wf_c = np.ascontiguousarray(wf[sl].T)               # [K, ns] bf16
        in_maps.append({"x": x, "wf": wf_c})
    return in_maps


_NC_CACHE = {}


def _get_nc(m, k, ns):
    key = (m, k, ns)
    if key not in _NC_CACHE:
        _NC_CACHE[key] = build_nc(m, k, ns)
    return _NC_CACHE[key]


def kernel(input, weight_scale, weight, group_size=GS, trace=False):
    m, k = input.shape
    n = weight.shape[0]
    ns = n // NCORES
    nc = _get_nc(m, k, ns)
    in_maps = host_prep(input, weight_scale, weight, ns)
    res = bass_utils.run_bass_kernel_spmd(
        nc, in_maps, core_ids=list(range(NCORES)), trace=trace
    )
    out = np.concatenate([r["out"] for r in res.results], axis=1)
    if trace:
        return out, res
    return out


if __name__ == "__main__":
    # small-config CoreSim check
    from concourse.bass_interp import CoreSim

    rng = np.random.default_rng(0)
    m, k, ns = 256, 2048, 256
    x = rng.standard_normal((m, k), dtype=np.float32)
    w_tern = rng.integers(-1, 2, size=(ns, k)).astype(np.int32)
    codes = (w_tern + 1).reshape(ns, k // 4, 4)
    packed = (
        codes[..., 0] | (codes[..., 1] << 2) | (codes[..., 2] << 4)
        | (codes[..., 3] << 6)
    ).astype(np.int32)
    ws = rng.uniform(0.001, 0.02, size=(ns, k // GS, 1)).astype(np.float32)

    # numpy reference
    s = 127.0 / np.clip(np.abs(x).max(axis=-1, keepdims=True), 1e-5, None)
    q = np.clip(np.round(x * s), -128, 127)
    wf = w_tern.astype(np.float32) * np.repeat(ws.reshape(ns, -1), GS, axis=1)
    ref = ((q @ wf.T) / s).astype(ml_dtypes.bfloat16).astype(np.float32)

    nc = build_nc(m, k, ns)
    im = host_prep(x, ws, packed, ns)[0]
    sim = CoreSim(nc)
    for kk, v in im.items():
        sim.tensor(kk)[:] = v
    sim.simulate()
    got = np.asarray(sim.tensor("out")).astype(np.float32)
    err = np.abs(got - ref).max() / (np.abs(ref).max() + 1e-9)
    print("rel err (absmax):", err)
    rms = np.sqrt(((got - ref) ** 2).mean()) / (np.sqrt((ref**2).mean()) + 1e-9)
    print("rel err (rms):", rms)


# revision 16
# speedup vs baseline: 1.2062x; 1.0205x over previous
"""BitLinear (ternary 2-bit weights, group-128 scales, dynamic int8 activation
quant) for Trainium2, tensor-parallel over 8 NeuronCores (shard N).

Math (per core, N-shard ns):
  s[m]   = 127 / clip(max_k |x[m,k]|, 1e-5)
  q[m,k] = round(x[m,k] * s[m])                      (integers in [-127,127])
  out[m,n] = (sum_k q[m,k] * w[n,k] * ws[n, k//128]) / s[m]   -> bf16

Device scheme ("stream-W" + int16 transposes):
- W is decoded host-side to full bf16 W[k,n] = (code-1)*ws (exact) and
  streamed from HBM; no on-device decode at all.
- Quantization is ONE activation pass: t1 = x*s + 2^23 (RNE rounding via the
  fp32 magic number).  t1's low 16 bits ARE q as int16 two's complement, so
  the PE transposes t1.bitcast(int16) stride-2 views directly
  (is_transpose=True, 1 cycle/row for 16-bit dtypes); the PSUM->SBUF evac
  converts int16 -> bf16.  The explicit "subtract 2^23" pass disappears.
- Mains: psm[mh][nh] += qT-slice.T @ W-tile over 64 k-blocks.
- Schedule: x first at full HBM bw (W gated on rowmax progress via dummy
  writes); resident W units (k-blocks 0..15) serve early mh0 mains (P1) and
  mh1 catch-up (P3, before P2 so both psm tiles finish together); units 4-15
  stream through a rotating pool for paired mains (P2).
- Dense fp32 dummy-matmul blocks tied to x-chunk arrivals keep the PE DVFS
  ramped through the head phase.
"""

import sys

import numpy as np

try:
    import concourse.bass as bass
except ImportError:  # fresh grading dir: fall back to the repo checkout
    sys.path.insert(0, "/opt/trn_rl_repo")
    import concourse.bass as bass

import ml_dtypes

import concourse.mybir as mybir
import concourse.tile as tile
from concourse import bacc, bass_utils
from concourse.masks import make_identity

FP32 = mybir.dt.float32
BF16 = mybir.dt.bfloat16
I16 = mybir.dt.int16
F16 = mybir.dt.float16
# 1.5 * 2^23: fp32 RNE rounds x*s to an integer AND every value in
# [1.5*2^23 - 128, 1.5*2^23 + 127] keeps exponent 150, so the low 16
# mantissa bits are exactly q in two's complement.
MAGIC = float(3 << 22)

M, N, K, GS = 256, 8192, 8192, 128
NCORES = 8


def build_nc(m=M, k=K, ns=N // NCORES):
    """One core's program: full m,k; n-shard of size ns."""
    mt = m // 128        # m partition-tiles (2)
    ck = k // 2048       # 2048-wide k-chunks for quant (4)
    kb = k // 128        # k-blocks / contraction tiles (64)
    npr = kb // 8        # transpose pairs: 8 k-blocks per PSUM bank (8)
    wu = kb // 4         # W DMA units of 4 k-blocks, 1MB each (16)
    n_res = min(4, wu)   # resident W units (k-blocks 0 .. 4*n_res-1)
    bres = 4 * n_res
    nsl = min(512, ns)   # matmul rhs free-dim slice (PSUM bank width)
    nh_n = ns // nsl

    nc = bacc.Bacc()
    x_d = nc.declare_dram_parameter("x", [m, k], FP32, isOutput=False)
    w_d = nc.declare_dram_parameter("wf", [k, ns], BF16, isOutput=False)
    out_d = nc.declare_dram_parameter("out", [m, ns], BF16, isOutput=True)

    x_r = x_d.rearrange("(T p) k -> T p k", p=128)            # [mt,128,k]
    w_r = w_d.rearrange("(u f p) n -> u p f n", f=4, p=128)   # [wu,128,4,ns]
    out_r = out_d.rearrange("(T p) n -> T p n", p=128)        # [mt,128,ns]

    with tile.TileContext(nc) as tc:
        with (
            tc.tile_pool(name="const", bufs=1) as constp,
            tc.tile_pool(name="stat", bufs=1) as statp,
            tc.tile_pool(name="qt", bufs=1) as qtp,
            tc.tile_pool(name="xp", bufs=8) as xp,
            tc.tile_pool(name="t1", bufs=4) as t1p,
            tc.tile_pool(name="wres", bufs=1) as wresp,
            tc.tile_pool(name="wstr", bufs=4) as wstrp,
            tc.tile_pool(name="ob", bufs=1) as obp,
            tc.tile_pool(name="psx", bufs=1, space="PSUM") as psxp,
            tc.tile_pool(name="pst", bufs=3, space="PSUM") as pstp,
            tc.tile_pool(name="psm", bufs=1, space="PSUM") as psmp,
        ):
            ident = constp.tile([128, 128], F16, tag="ident")
            make_identity(nc, ident)

            def warm(dep_tile, width, n_mm=1, name=""):
                """Dense fp32 dummy matmuls reading a landed tile: keep the
                PE DVFS/HAM ramped through the head phase."""
                for j in range(n_mm):
                    wp_ = psxp.tile([128, 512], FP32, tag="psx",
                                    name=f"wrm{name}{j}")
                    nc.tensor.matmul(
                        wp_[:, :width], dep_tile[:, :128],
                        dep_tile[:, :width],
                        start=True, stop=True,
                    )

            # t1 tiles (q in the low int16 of each fp32), per (mh, chunk)
            t1s = {}
            # transposed q pairs: [128, 8 kb-sub, 128 m] bf16 per (mh, pair)
            qtqp = {
                (mh, pr): qtp.tile([128, 1024], BF16, tag=f"qt{mh}_{pr}",
                                   name=f"qt{mh}_{pr}")
                for mh in range(mt) for pr in range(npr)
            }

            rpart = [statp.tile([128, ck], FP32, tag=f"rp{t}", name=f"rp{t}")
                     for t in range(mt)]
            rmax = [statp.tile([128, 1], FP32, tag=f"rm{t}", name=f"rm{t}")
                    for t in range(mt)]
            s_pp = [statp.tile([128, 1], FP32, tag=f"sp{t}", name=f"sp{t}")
                    for t in range(mt)]
            r1s = [statp.tile([128, 1], FP32, tag=f"rs{t}", name=f"rs{t}")
                   for t in range(mt)]

            psm = [
                [psmp.tile([128, nsl], FP32, tag=f"ps{mh}{nh}",
                           name=f"ps{mh}{nh}")
                 for nh in range(nh_n)]
                for mh in range(mt)
            ]

            wtiles = {}

            def load_w(u, pool, gate=None):
                wt = pool.tile([128, 4 * ns], BF16,
                               tag="wt" if pool is wstrp else f"wr{u}",
                               name=f"w{u}")
                if gate is not None:
                    # dummy write reading `gate`: delays this DMA until gate
                    # is produced (keeps W off the HBM while x streams)
                    nc.gpsimd.tensor_copy(wt[:1, :1], gate[:1, :1])
                wt3 = wt.rearrange("p (f n) -> p f n", f=4)
                nc.sync.dma_start(wt3[:], w_r[u])
                wtiles[u] = wt3

            def load_x(mh):
                xs = []
                for c in range(ck):
                    sl = slice(2048 * c, 2048 * (c + 1))
                    xc = xp.tile([128, 2048], FP32, tag="x", name=f"x{mh}{c}")
                    nc.sync.dma_start(xc[:], x_r[mh, :, sl])
                    xs.append(xc)
                return xs

            def rowmax_part(mh, c, xc):
                nc.vector.tensor_reduce(
                    rpart[mh][:, c : c + 1], xc[:],
                    axis=mybir.AxisListType.X, op=mybir.AluOpType.max,
                    apply_absolute_value=True,
                )

            def rowmax_fin(mh):
                nc.vector.tensor_reduce(
                    rmax[mh][:], rpart[mh][:],
                    axis=mybir.AxisListType.X, op=mybir.AluOpType.max,
                )
                nc.vector.tensor_scalar_max(rmax[mh][:], rmax[mh][:], 1e-5)
                nc.vector.reciprocal(s_pp[mh][:], rmax[mh][:])
                nc.vector.tensor_scalar_mul(s_pp[mh][:], s_pp[mh][:], 127.0)
                nc.vector.tensor_scalar_mul(r1s[mh][:], rmax[mh][:],
                                            1.0 / 127.0)

            def pass1(mh, c, xc):
                # t1 = x*s + 2^23: fp32 RNE puts q = round(x*s) in the low
                # mantissa bits; the int16 view of t1 IS q (two's complement).
                t1 = t1p.tile([128, 2048], FP32, tag="t1", name=f"t1_{mh}{c}")
                nc.scalar.activation(
                    t1[:], xc[:], mybir.ActivationFunctionType.Copy,
                    bias=MAGIC, scale=s_pp[mh][:],
                )
                t1s[(mh, c)] = t1

            def transpose_pair(mh, pr):
                """Transpose k-blocks 8pr..8pr+7 of m-tile mh into one PSUM
                bank via int16 is_transpose matmuls; return psT for evac."""
                psT = pstp.tile([128, 1024], F16, tag="psT")
                for j in range(8):
                    b = 8 * pr + j
                    c, jj = b // 16, b % 16
                    # fp16 bitcast view: the transpose is a raw bit move, the
                    # evac below reinterprets the bits as int16
                    qv = t1s[(mh, c)].bitcast(F16).rearrange(
                        "p (kk two) -> p kk two", two=2)[:, :, 0]
                    nc.tensor.transpose(
                        psT[:, 128 * j : 128 * (j + 1)],
                        qv[:, 128 * jj : 128 * (jj + 1)], ident[:],
                    )
                return psT

            def evac_pair(mh, pr, psT, eng):
                dst = qtqp[(mh, pr)][:]
                src_i16 = psT[:].bitcast(I16)
                if eng == 0:
                    nc.scalar.activation(
                        dst, src_i16, mybir.ActivationFunctionType.Copy
                    )
                else:
                    nc.vector.tensor_copy(dst, src_i16)

            started = [[False] * nh_n for _ in range(mt)]

            def mains(mh, b, stop=False):
                u = b // 4
                wt3 = wtiles[u]
                lhsT = qtqp[(mh, b // 8)].rearrange(
                    "p (f mm) -> p f mm", f=8)[:, b % 8, :]
                for nh in range(nh_n):
                    nc.tensor.matmul(
                        psm[mh][nh][:],
                        lhsT,
                        wt3[:, b % 4, nsl * nh : nsl * (nh + 1)],
                        start=not started[mh][nh], stop=stop,
                    )
                    started[mh][nh] = True

            def finalize(mh):
                # evac both psm tiles into one buffer (ACT), single DMA
                ob = obp.tile([128, ns], BF16, tag=f"ob{mh}", name=f"ob{mh}")
                for nh in range(nh_n):
                    nc.scalar.activation(
                        ob[:, nsl * nh : nsl * (nh + 1)], psm[mh][nh][:],
                        mybir.ActivationFunctionType.Copy, scale=r1s[mh][:],
                    )
                nc.scalar.dma_start(out_r[mh], ob[:])

            # ---------------- emission schedule ----------------
            xs0 = load_x(0)
            xs1 = load_x(1)     # both m-tiles stream at full bandwidth
            # PE warms tied to x chunk arrivals (fp32, dense) bridge the head
            for c in range(ck):
                warm(xs0[c], 512, n_mm=2 if c < ck - 1 else 4, name=f"a{c}")
            warm(xs1[0], 512, n_mm=2, name="b0")
            for c in range(ck):
                rowmax_part(0, c, xs0[c])
            rowmax_fin(0)
            # W residents gated so both x m-tiles keep HBM priority: u0 on
            # the last mt0 partial, the rest on mt1 rowmax progress
            load_w(0, wresp, gate=rpart[0][:, ck - 1 : ck])
            for c in range(ck):
                pass1(0, c, xs0[c])
            # mt0 pairs 0-1: transpose + DVE evac feed P1; first mt1 rowmax
            # chunks slot between them on the DVE queue
            psTa = transpose_pair(0, 0)
            psTb = transpose_pair(0, 1)
            rowmax_part(1, 0, xs1[0])
            if ck > 1:
                rowmax_part(1, 1, xs1[1])
            gc1 = min(1, ck - 1)
            for u in range(1, n_res):
                load_w(u, wresp, gate=rpart[1][:, gc1 : gc1 + 1])
            evac_pair(0, 0, psTa, eng=1)
            evac_pair(0, 1, psTb, eng=1)
            for c in range(2, ck):
                rowmax_part(1, c, xs1[c])
            rowmax_fin(1)
            # P1 (early mh0 mains on residents) interleaved with the rest of
            # the mt0 transposes; pairs 2-3 evac on ACT, 4+ on DVE (after the
            # mt1 rowmax chain in the DVE FIFO)
            for pr in range(2, npr):
                psT = transpose_pair(0, pr)
                evac_pair(0, pr, psT, eng=0 if pr < 4 else 1)
                lo, hi = 8 * (pr - 2), min(8 * (pr - 1), bres)
                for b in range(lo, hi):
                    mains(0, b, stop=(wu == n_res and b == kb - 1))
            for b in range(max(0, 8 * (npr - 2)), bres):
                mains(0, b, stop=(wu == n_res and b == kb - 1))
            for u in range(n_res, min(n_res + 4, wu)):
                load_w(u, wstrp, gate=rmax[1])
            warm(xs1[ck - 1], 512, n_mm=2, name="b3")
            # mt1 quant; transposes of pairs 0-1 feed P3 (mh1 catch-up on the
            # residents, before P2 so all psm tiles finish together)
            for c in range(ck):
                pass1(1, c, xs1[c])
            for pr in range(2):
                psT = transpose_pair(1, pr)
                evac_pair(1, pr, psT, eng=1)
            for b in range(0, bres):
                mains(1, b, stop=(wu == n_res and b == kb - 1))
            for pr in range(2, npr):
                psT = transpose_pair(1, pr)
                evac_pair(1, pr, psT, eng=1)
            for u in range(n_res + 4, wu):
                load_w(u, wstrp)
            # P2: paired mains on streamed W
            for b in range(bres, kb):
                mains(0, b, stop=(b == kb - 1))
                mains(1, b, stop=(b == kb - 1))
            finalize(0)
            finalize(1)
    nc.compile()
    return nc


def host_prep(input, weight_scale, weight, ns):
    """Shard + relayout inputs for each core: decode the packed 2-bit ternary
    codes and fold the per-(row, group) scale into a full bf16 W[k, n] per
    core (pure static-weight relayout), plus fp32 activation passthrough."""
    n, k4 = weight.shape
    k = k4 * 4
    x = np.ascontiguousarray(input, dtype=np.float32)
    w_bytes = weight.astype(np.uint8)                       # [N, K/4]
    codes = np.empty((n, k), dtype=np.int8)                 # [N, K] in {-1,0,1}
    for j in range(4):
        codes[:, j::4] = ((w_bytes >> (2 * j)) & 3).astype(np.int8) - 1
    ws2 = np.asarray(weight_scale, dtype=np.float32).reshape(n, -1)  # [N, K/GS]
    ws2_b = ws2.astype(ml_dtypes.bfloat16)
    # W[n, k] = codes * ws (exact in bf16: +-1 * bf16 scale)
    wf = codes.astype(np.float32) * ws2_b.astype(np.float32).repeat(GS, axis=1)
    wf = wf.astype(ml_dtypes.bfloat16)
    in_maps = []
    for c in range(n // ns):
        sl = slice(c * ns, (c + 1) * ns)
        wf_c = np.ascontiguousarray(wf[sl].T)               # [K, ns] bf16
        in_maps.append({"x": x, "wf": wf_c})
    return in_maps


_NC_CACHE = {}


def _get_nc(m, k, ns):
    key = (m, k, ns)
    if key not in _NC_CACHE:
        _NC_CACHE[key] = build_nc(m, k, ns)
    return _NC_CACHE[key]


def kernel(input, weight_scale, weight, group_size=GS, trace=False):
    m, k = input.shape
    n = weight.shape[0]
    ns = n // NCORES
    nc = _get_nc(m, k, ns)
    in_maps = host_prep(input, weight_scale, weight, ns)
    res = bass_utils.run_bass_kernel_spmd(
        nc, in_maps, core_ids=list(range(NCORES)), trace=trace
    )
    out = np.concatenate([r["out"] for r in res.results], axis=1)
    if trace:
        return out, res
    return out


if __name__ == "__main__":
    # small-config CoreSim check
    from concourse.bass_interp import CoreSim

    rng = np.random.default_rng(0)
    m, k, ns = 256, 2048, 256
    x = rng.standard_normal((m, k), dtype=np.float32)
    w_tern = rng.integers(-1, 2, size=(ns, k)).astype(np.int32)
    codes = (w_tern + 1).reshape(ns, k // 4, 4)
    packed = (
        codes[..., 0] | (codes[..., 1] << 2) | (codes[..., 2] << 4)
        | (codes[..., 3] << 6)
    ).astype(np.int32)
    ws = rng.uniform(0.001, 0.02, size=(ns, k // GS, 1)).astype(np.float32)

    # numpy reference
    s = 127.0 / np.clip(np.abs(x).max(axis=-1, keepdims=True), 1e-5, None)
    q = np.clip(np.round(x * s), -128, 127)
    wf = w_tern.astype(np.float32) * np.repeat(ws.reshape(ns, -1), GS, axis=1)
    ref = ((q @ wf.T) / s).astype(ml_dtypes.bfloat16).astype(np.float32)

    nc = build_nc(m, k, ns)
    im = host_prep(x, ws, packed, ns)[0]
    sim = CoreSim(nc)
    for kk, v in im.items():
        sim.tensor(kk)[:] = v
    sim.simulate()
    got = np.asarray(sim.tensor("out")).astype(np.float32)
    err = np.abs(got - ref).max() / (np.abs(ref).max() + 1e-9)
    print("rel err (absmax):", err)
    rms = np.sqrt(((got - ref) ** 2).mean()) / (np.sqrt((ref**2).mean()) + 1e-9)
    print("rel err (rms):", rms)
